# revision 24
# baseline (speedup 1.0000x reference)
"""Causal multi-head attention (B=2, S=2048, D=1024, H=16) on 8 TRN2 NeuronCores.

Sharding: core c handles batch b = c//4 and heads [4*(c%4), 4*(c%4)+4).
Each core computes its 4 heads' attention plus the partial w_o projection;
the host sums the 4 partials per batch (the "all-reduce after w_o") and
adds the w_o bias.

Compute dtype: bf16 matmul inputs with fp32 PSUM accumulation.

Layouts (per core, host-prepared):
  xT    [1024, 2048] bf16  x[b].T                    (d on partitions)
  wqk   [1024, 512]  bf16  cols = [k_h0..k_h3 | q_h0..q_h3] (64 each)
  wv    [1024, 256]  bf16  cols = [v_h0..v_h3]
  wo    [256, 1024]  bf16  w_o[:, head_cols].T
  bqk   [512, 1]     f32   per-feature bias, same col order as wqk
  bv    [256, 1]     f32   v bias per head
  fring [128, 128]   bf16  causal fringe pattern [k, q] = (q >= k); the only
                           masked cells of a diagonal 128-k-block are the
                           first 128 q columns at/after its offset.

In-kernel dataflow (per core):
  qkvT = wqk.T @ x.T  -> k/q in [feature, seq] layout, paired heads per tile
  v    = x @ wv       -> natural [seq, feature] + a ones column per head
  scores_T[k,q] = k_h.T(dk x 128) @ q_h(dk x 512)   (two heads row-tiled);
                  diagonal blocks stream only the causally-valid q columns
  p = exp(scores/8)   (ACT, psum->sbuf, bf16) on the valid span; the 128-col
                      fringe of each diagonal block is masked via DVE
  av_T[dk+1, q] = [v_h | 1].T @ p                   (row 64 = softmax denom)
  reciprocal of the denom row -> PE rank-1 broadcast (ones[1,64].T @ rec)
  avn = av * bc + bv  (DVE, -> bf16, [feature, seq] layout)
  y[s, o] = avn.T @ wo                              (partial, bf16 to HBM)
"""

import numpy as np
import ml_dtypes

import concourse.bass as bass
import concourse.mybir as mybir
import concourse.tile as tile
from concourse.bass_utils import run_bass_kernel_spmd
from concourse.vector_clock import ScopedClock

BF16 = mybir.dt.bfloat16
F32 = mybir.dt.float32
NP_BF16 = ml_dtypes.bfloat16

B, S, D = 2, 2048, 1024
H, DK = 16, 64
HPC = 4            # heads per core
N_CORES = 8
S_TILE = 512       # q tile width (f32 psum bank)
K_BLK = 128        # k block (partition dim of transposed scores)


# ---------------------------------------------------------------------------
# Workaround: this walrus build rejects >1 sem-wait on the TileContext exit
# Drain. Redistribute the global-clock waits onto single-wait sync NOPs.
# ---------------------------------------------------------------------------
def _patched_drain_and_barrier(self, tick_clock, wait_clock):
    probe = self.nc.sync.nop()
    wait_clock.add_sem_waits(probe.ins, ScopedClock({None: tick_clock.global_clock}))
    si = probe.ins.sync_info
    waits = list(si.on_wait)
    probe.ins.sync_info = mybir.SyncInfo(on_wait=waits[:1], on_update=list(si.on_update))
    for w in waits[1:]:
        nop = self.nc.sync.nop()
        nop.ins.sync_info = mybir.SyncInfo(on_wait=[w], on_update=[])
    self.nc.sync.drain()

    self.nc.all_engine_barrier()
    assert self.sems is not None
    popped = self.nc._tile_sem_poison_stack.pop()
    assert popped is self._sem_poison
    self.nc.clear_and_free_semaphores(list(self.sems.allocated().values()))
    self.nc.all_engine_barrier()


tile.TileContext._drain_and_barrier = _patched_drain_and_barrier

_CFG_SALT = "cfg-trim4"

_WAIT_LIMIT = 1


def _split_excess_waits(nc: bass.Bass, limit: int = _WAIT_LIMIT):
    """Walrus (this build) rejects instructions carrying more than a couple of
    sem waits. Move excess waits onto same-engine NOPs inserted just before."""
    n_split = 0
    for f in nc.m.functions:
        for bb in f.blocks:
            il = bb.instructions
            idx = 0
            while idx < len(il):
                inst = il[idx]
                si = inst.sync_info
                if si is not None and len(si.on_wait) > limit:
                    waits = list(si.on_wait)
                    pos = idx
                    for i in range(limit, len(waits), limit):
                        nop = mybir.InstNoOp(
                            name=f"{inst.name}_xw{i}", ins=[], outs=[]
                        )
                        nop.engine = inst.engine
                        nop.sync_info = mybir.SyncInfo(
                            on_wait=waits[i:i + limit], on_update=[]
                        )
                        il.insert(pos, nop)
                        pos += 1
                        idx += 1
                    inst.sync_info = mybir.SyncInfo(
                        on_wait=waits[:limit], on_update=list(si.on_update)
                    )
                    n_split += 1
                idx += 1
    return n_split


def build_attention_nc() -> bass.Bass:
    nc = bass.Bass("TRN2", target_bir_lowering=False, debug=False)

    xT_d = nc.dram_tensor("xT", [D, S], BF16, kind="ExternalInput").ap()
    wqk_d = nc.dram_tensor("wqk", [D, 8 * DK], BF16, kind="ExternalInput").ap()
    wv_d = nc.dram_tensor("wv", [D, 4 * DK], BF16, kind="ExternalInput").ap()
    wo_d = nc.dram_tensor("wo", [4 * DK, D], BF16, kind="ExternalInput").ap()
    bqk_d = nc.dram_tensor("bqk", [8 * DK, 1], F32, kind="ExternalInput").ap()
    bv_d = nc.dram_tensor("bv", [4 * DK, 1], F32, kind="ExternalInput").ap()
    fring_d = nc.dram_tensor("fring", [128, 128], BF16, kind="ExternalInput").ap()
    y_d = nc.dram_tensor("y", [S, D], BF16, kind="ExternalOutput").ap()

    n_kt = D // 128          # 8 contraction tiles over d
    n_st = S // 128          # 16 seq tiles of 128
    n_qt = S // S_TILE       # 4 q tiles of 512
    AV_LAG = 3               # AV trails exp by this many blocks in the stream

    from contextlib import ExitStack

    with tile.TileContext(nc) as tc, ExitStack() as stack:
        const = stack.enter_context(tc.tile_pool(name="const", bufs=1))
        xpool = stack.enter_context(tc.tile_pool(name="xp", bufs=1))
        kqpool = stack.enter_context(tc.tile_pool(name="kqp", bufs=1))
        vpool = stack.enter_context(tc.tile_pool(name="vp", bufs=1))
        avnpool = stack.enter_context(tc.tile_pool(name="avnp", bufs=1))
        ppool = stack.enter_context(tc.tile_pool(name="pp", bufs=24))
        spool = stack.enter_context(tc.tile_pool(name="sp", bufs=4))
        ypool = stack.enter_context(tc.tile_pool(name="yp", bufs=2))
        avsb = stack.enter_context(tc.tile_pool(name="avsb", bufs=4))
        # PSUM budget (8 banks): scores/y 2x[128,1024]=4, qk/v/bc 1, av 3.
        # bc shares the qkv bank (same tag/shape, rows 0:64 used) so the av
        # accumulators get a third bank to absorb head-pair switch stalls.
        sc_ps = stack.enter_context(tc.tile_pool(name="sc_ps", bufs=2, space="PSUM"))
        qkv_ps = stack.enter_context(tc.tile_pool(name="qkv_ps", bufs=1, space="PSUM"))
        av_ps = stack.enter_context(tc.tile_pool(name="av_ps", bufs=3, space="PSUM"))

        # --- resident loads (ordered so QKV compute can start early) ----
        xT, wqk, wv = [], [], []
        for i in range(n_kt):
            t = xpool.tile([128, S], BF16, tag=f"xT{i}", name=f"xT{i}")
            xT.append(t)
        for i in range(n_kt):
            w1 = const.tile([128, 8 * DK], BF16, tag=f"wqk{i}", name=f"wqk{i}")
            eng = (nc.scalar, nc.gpsimd)[i % 2]
            eng.dma_start(out=w1, in_=wqk_d[i * 128:(i + 1) * 128, :])
            wqk.append(w1)
            nc.sync.dma_start(
                out=xT[i][:, 0:S_TILE], in_=xT_d[i * 128:(i + 1) * 128, 0:S_TILE]
            )
        for i in range(n_kt):
            w2 = const.tile([128, 4 * DK], BF16, tag=f"wv{i}", name=f"wv{i}")
            nc.scalar.dma_start(out=w2, in_=wv_d[i * 128:(i + 1) * 128, :])
            wv.append(w2)
        for sq in range(1, n_qt):
            for i in range(n_kt):
                nc.sync.dma_start(
                    out=xT[i][:, sq * S_TILE:(sq + 1) * S_TILE],
                    in_=xT_d[i * 128:(i + 1) * 128, sq * S_TILE:(sq + 1) * S_TILE],
                )
        fring = const.tile([128, 128], BF16, tag="fring", name="fring")
        nc.gpsimd.dma_start(out=fring, in_=fring_d)
        bqk = []
        for i in range(4):
            t = const.tile([128, 1], F32, tag=f"bqk{i}", name=f"bqk{i}")
            nc.gpsimd.dma_start(out=t, in_=bqk_d[i * 128:(i + 1) * 128, :])
            bqk.append(t)
        bv = []
        for i in range(2):
            t = const.tile([128, 1], F32, tag=f"bv{i}", name=f"bv{i}")
            nc.gpsimd.dma_start(out=t, in_=bv_d[i * 128:(i + 1) * 128, :])
            bv.append(t)
        wo = []
        for i in range(2):
            t = const.tile([128, D], BF16, tag=f"wo{i}", name=f"wo{i}")
            nc.gpsimd.dma_start(out=t, in_=wo_d[i * 128:(i + 1) * 128, :])
            wo.append(t)
        ones_bf = const.tile([1, DK], BF16, tag="ones", name="ones")
        nc.vector.memset(ones_bf, 1.0)

        # kq[m][f, s]: m=0 -> k heads(0,1); 1 -> k heads(2,3); 2 -> q(0,1); 3 -> q(2,3)
        kq = [kqpool.tile([128, S], BF16, tag=f"kq{m}", name=f"kq{m}") for m in range(4)]
        # v_sb[st][128, 4*65]: per head h: cols [h*65, h*65+64) = v, col h*65+64 = 1.0
        v_sb = [vpool.tile([128, HPC * (DK + 1)], BF16, tag=f"v{st}", name=f"v{st}")
                for st in range(n_st)]
        # avn[f2][f, s]: f2=0 -> heads (0,1); f2=1 -> heads (2,3)
        avn = [avnpool.tile([128, S], BF16, tag=f"avn{f2}", name=f"avn{f2}")
               for f2 in range(2)]

        def emit_kq(m, sq):
            ps = qkv_ps.tile([128, S_TILE], F32, tag="qkps", name="qkps")
            for kt in range(n_kt):
                nc.tensor.matmul(
                    ps,
                    lhsT=wqk[kt][:, m * 128:(m + 1) * 128],
                    rhs=xT[kt][:, sq * S_TILE:(sq + 1) * S_TILE],
                    start=(kt == 0),
                    stop=(kt == n_kt - 1),
                )
            nc.vector.tensor_scalar_add(
                kq[m][:, sq * S_TILE:(sq + 1) * S_TILE], ps, bqk[m]
            )

        def emit_v(st):
            # Shares the qkps tag/bank (cols 256+ unused) so bc_ps fits in
            # the 8-bank PSUM budget.
            ps_full = qkv_ps.tile([128, S_TILE], F32, tag="qkps", name="vps")
            ps = ps_full[:, 0:HPC * DK]
            for kt in range(n_kt):
                nc.tensor.matmul(
                    ps,
                    lhsT=xT[kt][:, st * 128:(st + 1) * 128],
                    rhs=wv[kt],
                    start=(kt == 0),
                    stop=(kt == n_kt - 1),
                )
            nc.vector.memset(v_sb[st], 1.0)
            for h in range(HPC):
                nc.vector.tensor_copy(
                    out=v_sb[st][:, h * (DK + 1):h * (DK + 1) + DK],
                    in_=ps[:, h * DK:(h + 1) * DK],
                )

        def emit_wo(st):
            yp = sc_ps.tile([128, D], F32, tag="scps", name="yps")
            for oh in range(2):
                for f2 in range(2):
                    nc.tensor.matmul(
                        yp[:, oh * 512:(oh + 1) * 512],
                        lhsT=avn[f2][:, st * 128:(st + 1) * 128],
                        rhs=wo[f2][:, oh * 512:(oh + 1) * 512],
                        start=(f2 == 0),
                        stop=(f2 == 1),
                    )
            y_sb = ypool.tile([128, D], BF16, tag="ysb", name="ysb")
            nc.vector.tensor_copy(out=y_sb, in_=yp)
            nc.sync.dma_start(out=y_d[st * 128:(st + 1) * 128, :], in_=y_sb)

        def qkv_round(sq):
            return [
                lambda m=m, sq=sq: emit_kq(m, sq) for m in (0, 2, 1, 3)
            ] + [lambda st=st: emit_v(st) for st in range(4 * sq, 4 * sq + 4)]

        def make_normalize(t, hp, av_c):
            """Deferred: reciprocal of the denom row, PE rank-1 broadcast,
            then avn = av * (1/den) + bv. Deferring keeps the PE's in-order
            queue from parking on the DVE reciprocal chain."""
            def run():
                for i in range(2):
                    # 1/den as exp(-ln(den)) on ACT: DVE reciprocal on a
                    # single-partition [1, 512] row costs ~3.3us (free-dim
                    # serial) and parks the whole DVE queue; ACT runs the two
                    # table ops in ~1.6us off the DVE/PE critical path.
                    lnd = spool.tile([1, S_TILE], F32, tag="rec", name="rec")
                    nc.scalar.activation(lnd, av_c[i][DK:DK + 1, :],
                                         mybir.ActivationFunctionType.Ln)
                    rec_bf = spool.tile([1, S_TILE], BF16, tag="recbf", name="recbf")
                    nc.scalar.activation(rec_bf, lnd,
                                         mybir.ActivationFunctionType.Exp,
                                         scale=-1.0)
                    bc_full = qkv_ps.tile([128, S_TILE], F32, tag="qkps",
                                          name="bc")
                    bc = bc_full[0:DK, :]
                    nc.tensor.matmul(bc, lhsT=ones_bf, rhs=rec_bf, start=True,
                                     stop=True)
                    if i == 0:
                        dst = avn[hp][0:DK, t * S_TILE:(t + 1) * S_TILE]
                        nc.vector.tensor_mul(dst, av_c[i][0:DK, :], bc)
                        nc.vector.tensor_scalar_add(dst, dst, bv[hp][0:DK, :])
                    else:
                        tmp = spool.tile([DK, S_TILE], BF16, tag="avtmp",
                                         name="avtmp")
                        nc.vector.tensor_mul(tmp, av_c[i][0:DK, :], bc)
                        nc.vector.tensor_scalar_add(tmp, tmp, bv[hp][64:64 + DK, :])
                        nc.gpsimd.dma_start(
                            out=avn[hp][64:128, t * S_TILE:(t + 1) * S_TILE],
                            in_=tmp,
                        )
            return run

        def attention_tile(t, jobs):
            """Emit attention for q-tile t, interleaving `jobs` (QKV groups of
            the next round, w_o of the previous tile) into the stream. AV
            matmuls trail their exp by AV_LAG blocks so the in-order PE
            stream never parks on an unfinished exp. Diagonal blocks stream
            only their causally-valid q columns; the 128-col fringe is masked.
            Returns the deferred normalize job for the hp=1 half."""
            nblk = 4 * t + 4
            stride = max(1, (2 * nblk) // max(1, len(jobs)))
            s = 0
            last_norm = None
            for hp in range(2):
                kt2 = kq[hp]
                qt2 = kq[2 + hp]
                av_t = [av_ps.tile([128, S_TILE], F32, tag="avps", name="avps")
                        for _ in range(2)]
                pend = []

                def emit_av(blk, p):
                    dd = blk - 4 * t
                    q_off = max(0, dd) * K_BLK
                    for i in range(2):
                        h = 2 * hp + i
                        nc.tensor.matmul(
                            av_t[i][0:DK + 1, q_off:S_TILE],
                            lhsT=v_sb[blk][:, h * (DK + 1):(h + 1) * (DK + 1)],
                            rhs=p[:, i * S_TILE + q_off:(i + 1) * S_TILE],
                            start=(blk == 0),
                            stop=(blk == nblk - 1),
                            skip_group_check=True,
                        )

                for blk in range(nblk):
                    if jobs and s % stride == 0:
                        jobs.pop(0)()
                    s += 1
                    dd = blk - 4 * t
                    q_off = max(0, dd) * K_BLK
                    sc = sc_ps.tile([128, 2 * S_TILE], F32, tag="scps", name="scps")
                    for i in range(2):  # head A / head B, row-tiled pair
                        nc.tensor.matmul(
                            sc[:, i * S_TILE + q_off:(i + 1) * S_TILE],
                            lhsT=kt2[i * 64:(i + 1) * 64, blk * K_BLK:(blk + 1) * K_BLK],
                            rhs=qt2[i * 64:(i + 1) * 64,
                                    t * S_TILE + q_off:(t + 1) * S_TILE],
                            start=True,
                            stop=True,
                            tile_position=(i * 64, 0),
                        )
                    p = ppool.tile([128, 2 * S_TILE], BF16, tag="p", name="p")
                    # One instruction over [q_off, 1024): the gap between the
                    # two heads' valid spans exps stale-but-finite psum (never
                    # read downstream); ACT fixed cost ~300ns makes one wide
                    # instruction cheaper than two narrow ones.
                    nc.scalar.activation(
                        p[:, q_off:], sc[:, q_off:],
                        mybir.ActivationFunctionType.Exp, scale=0.125,
                    )
                    if dd >= 0:       # diagonal block: mask the 128-col fringe
                        for i in range(2):
                            a = i * S_TILE + q_off
                            nc.vector.tensor_mul(
                                p[:, a:a + K_BLK], p[:, a:a + K_BLK], fring
                            )
                    pend.append((blk, p))
                    if len(pend) > AV_LAG:
                        emit_av(*pend.pop(0))
                while pend:
                    if jobs and s % stride == 0:
                        jobs.pop(0)()
                    s += 1
                    emit_av(*pend.pop(0))
                # move av (+denominator row) off PSUM right away
                av_c = []
                for i in range(2):
                    c = avsb.tile([DK + 1, S_TILE], F32, tag="avc", name="avc")
                    nc.vector.tensor_copy(out=c, in_=av_t[i][0:DK + 1, :])
                    av_c.append(c)
                norm = make_normalize(t, hp, av_c)
                if hp == 0:
                    jobs.append(norm)   # consumed inside hp=1's stream
                else:
                    last_norm = norm    # handed to the next tile's job list
            while jobs:
                jobs.pop(0)()
            return last_norm

        for job in qkv_round(0):
            job()
        norm1 = None
        for t in range(n_qt):
            jobs = list(qkv_round(t + 1)) if t + 1 < n_qt else []
            if t > 0:
                jobs = [lambda st=st: emit_wo(st) for st in range(4 * (t - 1), 4 * t)] + jobs
            if norm1 is not None:
                jobs = [norm1] + jobs
            norm1 = attention_tile(t, jobs)
        norm1()
        for st in range(4 * (n_qt - 1), n_st):
            emit_wo(st)

    _split_excess_waits(nc)
    salt = mybir.InstNoOp(name=f"salt_{_CFG_SALT}", ins=[], outs=[])
    salt.engine = mybir.EngineType.SP
    nc.m.functions[0].blocks[0].instructions.insert(0, salt)
    return nc


_CACHED_NC = None


def _get_nc():
    global _CACHED_NC
    if _CACHED_NC is None:
        _CACHED_NC = build_attention_nc()
    return _CACHED_NC


def _prep_core_inputs(x, mask, w_qkv_w, w_qkv_b, w_o_w, w_o_b, core):
    b = core // 4
    hg = core % 4
    heads = [hg * HPC + h for h in range(HPC)]

    xT = np.ascontiguousarray(x[b].T).astype(NP_BF16)

    def rows(sec, h):  # q=0, k=1, v=2
        base = sec * D + h * DK
        return slice(base, base + DK)

    wqk_rows = np.concatenate(
        [w_qkv_w[rows(1, h)] for h in heads] + [w_qkv_w[rows(0, h)] for h in heads],
        axis=0,
    )  # [512, 1024]
    wqk = np.ascontiguousarray(wqk_rows.T).astype(NP_BF16)

    wv_rows = np.concatenate([w_qkv_w[rows(2, h)] for h in heads], axis=0)
    wv = np.ascontiguousarray(wv_rows.T).astype(NP_BF16)

    wo = np.ascontiguousarray(
        w_o_w[:, hg * HPC * DK:(hg + 1) * HPC * DK].T
    ).astype(NP_BF16)

    bqk = np.concatenate(
        [w_qkv_b[rows(1, h)] for h in heads] + [w_qkv_b[rows(0, h)] for h in heads]
    ).astype(np.float32)[:, None]
    bv = np.concatenate([w_qkv_b[rows(2, h)] for h in heads]).astype(np.float32)[:, None]

    # Causal fringe pattern from the provided mask tensor: for the exact
    # diagonal 128x128 sub-block, [k, q] = mask[q, k] (1 iff q >= k).
    m2d = np.asarray(mask[0, 0])
    fring = np.ascontiguousarray(m2d[0:K_BLK, 0:K_BLK].T.astype(np.float32)).astype(
        NP_BF16
    )

    return {
        "xT": xT, "wqk": wqk, "wv": wv, "wo": wo,
        "bqk": bqk, "bv": bv, "fring": fring,
    }


def kernel(x, mask, w_qkv_w, w_qkv_b, w_o_w, w_o_b, _profile=False):
    x = np.asarray(x, np.float32)
    w_qkv_w = np.asarray(w_qkv_w, np.float32)
    w_qkv_b = np.asarray(w_qkv_b, np.float32)
    w_o_w = np.asarray(w_o_w, np.float32)
    w_o_b = np.asarray(w_o_b, np.float32)

    nc = _get_nc()
    in_maps = [
        _prep_core_inputs(x, mask, w_qkv_w, w_qkv_b, w_o_w, w_o_b, c)
        for c in range(N_CORES)
    ]
    res = run_bass_kernel_spmd(
        nc, in_maps, core_ids=list(range(N_CORES)), trace=_profile
    )
    y = np.zeros((B, S, D), np.float32)
    for c in range(N_CORES):
        y[c // 4] += np.asarray(res.results[c]["y"], np.float32)
    y += w_o_b[None, None, :]
    if _profile:
        return y, res
    return y


# revision 28
# speedup vs baseline: 1.0817x; 1.0817x over previous
"""Causal multi-head attention (B=2, S=2048, D=1024, H=16) on 8 TRN2 NeuronCores.

Sharding: core c handles batch b = c//4 and heads [4*(c%4), 4*(c%4)+4).
Each core computes its 4 heads' attention plus the partial w_o projection;
the host sums the 4 partials per batch (the "all-reduce after w_o") and
adds the w_o bias.

Compute dtype: bf16 matmul inputs with fp32 PSUM accumulation.

Layouts (per core, host-prepared):
  xT    [1024, 2048] bf16  x[b].T                    (d on partitions)
  wqk   [1024, 512]  bf16  cols = [k_h0..k_h3 | q_h0..q_h3] (64 each)
  wv    [1024, 256]  bf16  cols = [v_h0..v_h3]
  wo    [256, 1024]  bf16  w_o[:, head_cols].T
  bqk   [512, 1]     f32   per-feature bias, same col order as wqk
  bv    [256, 1]     f32   v bias per head
  fring [128, 128]   bf16  causal fringe pattern [k, q] = (q >= k); the only
                           masked cells of a diagonal 128-k-block are the
                           first 128 q columns at/after its offset.

In-kernel dataflow (per core):
  qkvT = wqk.T @ x.T  -> k/q in [feature, seq] layout, paired heads per tile
  v    = x @ wv       -> natural [seq, feature] + a ones column per head
  scores_T[k,q] = k_h.T(dk x 128) @ q_h(dk x 512)   (two heads row-tiled);
                  diagonal blocks stream only the causally-valid q columns
  p = exp(scores/8)   (ACT, psum->sbuf, bf16) on the valid span; the 128-col
                      fringe of each diagonal block is masked via DVE
  av_T[dk+1, q] = [v_h | 1].T @ p                   (row 64 = softmax denom)
  reciprocal of the denom row -> PE rank-1 broadcast (ones[1,64].T @ rec)
  avn = av * bc + bv  (DVE, -> bf16, [feature, seq] layout)
  y[s, o] = avn.T @ wo                              (partial, bf16 to HBM)
"""

import numpy as np
import ml_dtypes

import concourse.bass as bass
import concourse.mybir as mybir
import concourse.tile as tile
from concourse.bass_utils import run_bass_kernel_spmd
from concourse.vector_clock import ScopedClock

BF16 = mybir.dt.bfloat16
F32 = mybir.dt.float32
NP_BF16 = ml_dtypes.bfloat16

B, S, D = 2, 2048, 1024
H, DK = 16, 64
HPC = 4            # heads per core
N_CORES = 8
S_TILE = 512       # q tile width (f32 psum bank)
K_BLK = 128        # k block (partition dim of transposed scores)


# ---------------------------------------------------------------------------
# Workaround: this walrus build rejects >1 sem-wait on the TileContext exit
# Drain. Redistribute the global-clock waits onto single-wait sync NOPs.
# ---------------------------------------------------------------------------
def _patched_drain_and_barrier(self, tick_clock, wait_clock):
    probe = self.nc.sync.nop()
    wait_clock.add_sem_waits(probe.ins, ScopedClock({None: tick_clock.global_clock}))
    si = probe.ins.sync_info
    waits = list(si.on_wait)
    probe.ins.sync_info = mybir.SyncInfo(on_wait=waits[:1], on_update=list(si.on_update))
    for w in waits[1:]:
        nop = self.nc.sync.nop()
        nop.ins.sync_info = mybir.SyncInfo(on_wait=[w], on_update=[])
    self.nc.sync.drain()

    self.nc.all_engine_barrier()
    assert self.sems is not None
    popped = self.nc._tile_sem_poison_stack.pop()
    assert popped is self._sem_poison
    self.nc.clear_and_free_semaphores(list(self.sems.allocated().values()))
    self.nc.all_engine_barrier()


tile.TileContext._drain_and_barrier = _patched_drain_and_barrier

_CFG_SALT = "cfg-trim5"

_WAIT_LIMIT = 1


def _split_excess_waits(nc: bass.Bass, limit: int = _WAIT_LIMIT):
    """Walrus (this build) rejects instructions carrying more than a couple of
    sem waits. Move excess waits onto same-engine NOPs inserted just before."""
    n_split = 0
    for f in nc.m.functions:
        for bb in f.blocks:
            il = bb.instructions
            idx = 0
            while idx < len(il):
                inst = il[idx]
                si = inst.sync_info
                if si is not None and len(si.on_wait) > limit:
                    waits = list(si.on_wait)
                    pos = idx
                    for i in range(limit, len(waits), limit):
                        nop = mybir.InstNoOp(
                            name=f"{inst.name}_xw{i}", ins=[], outs=[]
                        )
                        nop.engine = inst.engine
                        nop.sync_info = mybir.SyncInfo(
                            on_wait=waits[i:i + limit], on_update=[]
                        )
                        il.insert(pos, nop)
                        pos += 1
                        idx += 1
                    inst.sync_info = mybir.SyncInfo(
                        on_wait=waits[:limit], on_update=list(si.on_update)
                    )
                    n_split += 1
                idx += 1
    return n_split


def build_attention_nc() -> bass.Bass:
    nc = bass.Bass("TRN2", target_bir_lowering=False, debug=False)

    xT_d = nc.dram_tensor("xT", [D, S], BF16, kind="ExternalInput").ap()
    wqk_d = nc.dram_tensor("wqk", [D, 8 * DK], BF16, kind="ExternalInput").ap()
    wv_d = nc.dram_tensor("wv", [D, 4 * DK], BF16, kind="ExternalInput").ap()
    wo_d = nc.dram_tensor("wo", [4 * DK, D], BF16, kind="ExternalInput").ap()
    bqk_d = nc.dram_tensor("bqk", [8 * DK, 1], F32, kind="ExternalInput").ap()
    bv_d = nc.dram_tensor("bv", [4 * DK, 1], F32, kind="ExternalInput").ap()
    fring_d = nc.dram_tensor("fring", [128, 128], BF16, kind="ExternalInput").ap()
    y_d = nc.dram_tensor("y", [S, D], BF16, kind="ExternalOutput").ap()

    n_kt = D // 128          # 8 contraction tiles over d
    n_st = S // 128          # 16 seq tiles of 128
    n_qt = S // S_TILE       # 4 q tiles of 512
    AV_LAG = 2               # AV trails exp by this many blocks in the stream

    from contextlib import ExitStack

    with tile.TileContext(nc) as tc, ExitStack() as stack:
        const = stack.enter_context(tc.tile_pool(name="const", bufs=1))
        xpool = stack.enter_context(tc.tile_pool(name="xp", bufs=1))
        kqpool = stack.enter_context(tc.tile_pool(name="kqp", bufs=1))
        vpool = stack.enter_context(tc.tile_pool(name="vp", bufs=1))
        avnpool = stack.enter_context(tc.tile_pool(name="avnp", bufs=1))
        ppool = stack.enter_context(tc.tile_pool(name="pp", bufs=24))
        spool = stack.enter_context(tc.tile_pool(name="sp", bufs=4))
        ypool = stack.enter_context(tc.tile_pool(name="yp", bufs=2))
        avsb = stack.enter_context(tc.tile_pool(name="avsb", bufs=4))
        # PSUM budget (8 banks): scores/y 2x[128,1024]=4, qk/v 1, av 2, bc 1.
        sc_ps = stack.enter_context(tc.tile_pool(name="sc_ps", bufs=2, space="PSUM"))
        qkv_ps = stack.enter_context(tc.tile_pool(name="qkv_ps", bufs=1, space="PSUM"))
        av_ps = stack.enter_context(tc.tile_pool(name="av_ps", bufs=2, space="PSUM"))
        bc_ps = stack.enter_context(tc.tile_pool(name="bc_ps", bufs=1, space="PSUM"))

        # --- resident loads (ordered so QKV compute can start early) ----
        xT, wqk, wv = [], [], []
        for i in range(n_kt):
            t = xpool.tile([128, S], BF16, tag=f"xT{i}", name=f"xT{i}")
            xT.append(t)
        for i in range(n_kt):
            w1 = const.tile([128, 8 * DK], BF16, tag=f"wqk{i}", name=f"wqk{i}")
            eng = (nc.scalar, nc.gpsimd)[i % 2]
            eng.dma_start(out=w1, in_=wqk_d[i * 128:(i + 1) * 128, :])
            wqk.append(w1)
            nc.sync.dma_start(
                out=xT[i][:, 0:S_TILE], in_=xT_d[i * 128:(i + 1) * 128, 0:S_TILE]
            )
        for i in range(n_kt):
            w2 = const.tile([128, 4 * DK], BF16, tag=f"wv{i}", name=f"wv{i}")
            nc.scalar.dma_start(out=w2, in_=wv_d[i * 128:(i + 1) * 128, :])
            wv.append(w2)
        for sq in range(1, n_qt):
            for i in range(n_kt):
                nc.sync.dma_start(
                    out=xT[i][:, sq * S_TILE:(sq + 1) * S_TILE],
                    in_=xT_d[i * 128:(i + 1) * 128, sq * S_TILE:(sq + 1) * S_TILE],
                )
        fring = const.tile([128, 128], BF16, tag="fring", name="fring")
        nc.gpsimd.dma_start(out=fring, in_=fring_d)
        bqk = []
        for i in range(4):
            t = const.tile([128, 1], F32, tag=f"bqk{i}", name=f"bqk{i}")
            nc.gpsimd.dma_start(out=t, in_=bqk_d[i * 128:(i + 1) * 128, :])
            bqk.append(t)
        bv = []
        for i in range(2):
            t = const.tile([128, 1], F32, tag=f"bv{i}", name=f"bv{i}")
            nc.gpsimd.dma_start(out=t, in_=bv_d[i * 128:(i + 1) * 128, :])
            bv.append(t)
        wo = []
        for i in range(2):
            t = const.tile([128, D], BF16, tag=f"wo{i}", name=f"wo{i}")
            nc.gpsimd.dma_start(out=t, in_=wo_d[i * 128:(i + 1) * 128, :])
            wo.append(t)
        ones_bf = const.tile([1, DK], BF16, tag="ones", name="ones")
        nc.vector.memset(ones_bf, 1.0)

        # kq[m][f, s]: m=0 -> k heads(0,1); 1 -> k heads(2,3); 2 -> q(0,1); 3 -> q(2,3)
        kq = [kqpool.tile([128, S], BF16, tag=f"kq{m}", name=f"kq{m}") for m in range(4)]
        # v_sb[st][128, 4*65]: per head h: cols [h*65, h*65+64) = v, col h*65+64 = 1.0
        v_sb = [vpool.tile([128, HPC * (DK + 1)], BF16, tag=f"v{st}", name=f"v{st}")
                for st in range(n_st)]
        # avn[f2][f, s]: f2=0 -> heads (0,1); f2=1 -> heads (2,3)
        avn = [avnpool.tile([128, S], BF16, tag=f"avn{f2}", name=f"avn{f2}")
               for f2 in range(2)]

        def emit_kq(m, sq):
            ps = qkv_ps.tile([128, S_TILE], F32, tag="qkps", name="qkps")
            for kt in range(n_kt):
                nc.tensor.matmul(
                    ps,
                    lhsT=wqk[kt][:, m * 128:(m + 1) * 128],
                    rhs=xT[kt][:, sq * S_TILE:(sq + 1) * S_TILE],
                    start=(kt == 0),
                    stop=(kt == n_kt - 1),
                )
            nc.vector.tensor_scalar_add(
                kq[m][:, sq * S_TILE:(sq + 1) * S_TILE], ps, bqk[m]
            )

        def emit_v(st):
            # Shares the qkps tag/bank (cols 256+ unused) so bc_ps fits in
            # the 8-bank PSUM budget.
            ps_full = qkv_ps.tile([128, S_TILE], F32, tag="qkps", name="vps")
            ps = ps_full[:, 0:HPC * DK]
            for kt in range(n_kt):
                nc.tensor.matmul(
                    ps,
                    lhsT=xT[kt][:, st * 128:(st + 1) * 128],
                    rhs=wv[kt],
                    start=(kt == 0),
                    stop=(kt == n_kt - 1),
                )
            nc.vector.memset(v_sb[st], 1.0)
            for h in range(HPC):
                nc.vector.tensor_copy(
                    out=v_sb[st][:, h * (DK + 1):h * (DK + 1) + DK],
                    in_=ps[:, h * DK:(h + 1) * DK],
                )

        def emit_wo(st):
            yp = sc_ps.tile([128, D], F32, tag="scps", name="yps")
            for oh in range(2):
                for f2 in range(2):
                    nc.tensor.matmul(
                        yp[:, oh * 512:(oh + 1) * 512],
                        lhsT=avn[f2][:, st * 128:(st + 1) * 128],
                        rhs=wo[f2][:, oh * 512:(oh + 1) * 512],
                        start=(f2 == 0),
                        stop=(f2 == 1),
                    )
            y_sb = ypool.tile([128, D], BF16, tag="ysb", name="ysb")
            nc.vector.tensor_copy(out=y_sb, in_=yp)
            nc.sync.dma_start(out=y_d[st * 128:(st + 1) * 128, :], in_=y_sb)

        def qkv_round(sq):
            return [
                lambda m=m, sq=sq: emit_kq(m, sq) for m in (0, 2, 1, 3)
            ] + [lambda st=st: emit_v(st) for st in range(4 * sq, 4 * sq + 4)]

        def make_normalize(t, hp, av_c):
            """Deferred: reciprocal of the denom row, PE rank-1 broadcast,
            then avn = av * (1/den) + bv. Deferring keeps the PE's in-order
            queue from parking on the DVE reciprocal chain."""
            def run():
                # 1/den as exp(-ln(den)) on ACT: DVE reciprocal on a
                # single-partition row costs ~6.5ns/elem (free-dim serial)
                # and parks the whole DVE queue; ACT runs the two table ops
                # off the DVE/PE critical path, both heads in one pass.
                lnd = spool.tile([1, 2 * S_TILE], F32, tag="rec", name="rec")
                nc.scalar.activation(lnd, av_c[DK:DK + 1, :],
                                     mybir.ActivationFunctionType.Ln)
                rec_bf = spool.tile([1, 2 * S_TILE], BF16, tag="recbf",
                                    name="recbf")
                nc.scalar.activation(rec_bf, lnd,
                                     mybir.ActivationFunctionType.Exp,
                                     scale=-1.0)
                for i in range(2):
                    bc = bc_ps.tile([DK, S_TILE], F32, tag="bc", name="bc")
                    nc.tensor.matmul(
                        bc, lhsT=ones_bf,
                        rhs=rec_bf[:, i * S_TILE:(i + 1) * S_TILE],
                        start=True, stop=True,
                    )
                    av_i = av_c[0:DK, i * S_TILE:(i + 1) * S_TILE]
                    if i == 0:
                        dst = avn[hp][0:DK, t * S_TILE:(t + 1) * S_TILE]
                        nc.vector.tensor_mul(dst, av_i, bc)
                        nc.vector.tensor_scalar_add(dst, dst, bv[hp][0:DK, :])
                    else:
                        tmp = spool.tile([DK, S_TILE], BF16, tag="avtmp",
                                         name="avtmp")
                        nc.vector.tensor_mul(tmp, av_i, bc)
                        nc.vector.tensor_scalar_add(tmp, tmp, bv[hp][64:64 + DK, :])
                        nc.gpsimd.dma_start(
                            out=avn[hp][64:128, t * S_TILE:(t + 1) * S_TILE],
                            in_=tmp,
                        )
            return run

        def attention_tile(t, jobs):
            """Emit attention for q-tile t, interleaving `jobs` (QKV groups of
            the next round, w_o of the previous tile) into the stream. AV
            matmuls trail their exp by AV_LAG blocks so the in-order PE
            stream never parks on an unfinished exp. Diagonal blocks stream
            only their causally-valid q columns; the 128-col fringe is masked.
            Returns the deferred normalize job for the hp=1 half."""
            nblk = 4 * t + 4
            stride = max(1, (2 * nblk) // max(1, len(jobs)))
            s = 0
            last_norm = None
            for hp in range(2):
                kt2 = kq[hp]
                qt2 = kq[2 + hp]
                av_t = [av_ps.tile([128, S_TILE], F32, tag="avps", name="avps")
                        for _ in range(2)]
                pend = []

                def emit_av(blk, p):
                    dd = blk - 4 * t
                    q_off = max(0, dd) * K_BLK
                    for i in range(2):
                        h = 2 * hp + i
                        nc.tensor.matmul(
                            av_t[i][0:DK + 1, q_off:S_TILE],
                            lhsT=v_sb[blk][:, h * (DK + 1):(h + 1) * (DK + 1)],
                            rhs=p[:, i * S_TILE + q_off:(i + 1) * S_TILE],
                            start=(blk == 0),
                            stop=(blk == nblk - 1),
                            skip_group_check=True,
                        )

                for blk in range(nblk):
                    if jobs and s % stride == 0:
                        jobs.pop(0)()
                    s += 1
                    dd = blk - 4 * t
                    q_off = max(0, dd) * K_BLK
                    sc = sc_ps.tile([128, 2 * S_TILE], F32, tag="scps", name="scps")
                    for i in range(2):  # head A / head B, row-tiled pair
                        nc.tensor.matmul(
                            sc[:, i * S_TILE + q_off:(i + 1) * S_TILE],
                            lhsT=kt2[i * 64:(i + 1) * 64, blk * K_BLK:(blk + 1) * K_BLK],
                            rhs=qt2[i * 64:(i + 1) * 64,
                                    t * S_TILE + q_off:(t + 1) * S_TILE],
                            start=True,
                            stop=True,
                            tile_position=(i * 64, 0),
                        )
                    p = ppool.tile([128, 2 * S_TILE], BF16, tag="p", name="p")
                    # One instruction over [q_off, 1024): the gap between the
                    # two heads' valid spans exps stale-but-finite psum (never
                    # read downstream); ACT fixed cost ~300ns makes one wide
                    # instruction cheaper than two narrow ones.
                    nc.scalar.activation(
                        p[:, q_off:], sc[:, q_off:],
                        mybir.ActivationFunctionType.Exp, scale=0.125,
                    )
                    if dd >= 0:       # diagonal block: mask the 128-col fringe
                        for i in range(2):
                            a = i * S_TILE + q_off
                            nc.vector.tensor_mul(
                                p[:, a:a + K_BLK], p[:, a:a + K_BLK], fring
                            )
                    pend.append((blk, p))
                    if len(pend) > AV_LAG:
                        emit_av(*pend.pop(0))
                while pend:
                    if jobs and s % stride == 0:
                        jobs.pop(0)()
                    s += 1
                    emit_av(*pend.pop(0))
                # move av (+denominator row) off PSUM right away; both heads
                # land in one [65, 1024] tile so the denominator ln/exp runs
                # once over [1, 1024] instead of twice over [1, 512]
                av_c = avsb.tile([DK + 1, 2 * S_TILE], F32, tag="avc", name="avc")
                for i in range(2):
                    nc.vector.tensor_copy(
                        out=av_c[:, i * S_TILE:(i + 1) * S_TILE],
                        in_=av_t[i][0:DK + 1, :],
                    )
                norm = make_normalize(t, hp, av_c)
                if hp == 0:
                    jobs.append(norm)   # consumed inside hp=1's stream
                else:
                    last_norm = norm    # handed to the next tile's job list
            while jobs:
                jobs.pop(0)()
            return last_norm

        for job in qkv_round(0):
            job()
        norm1 = None
        for t in range(n_qt):
            jobs = list(qkv_round(t + 1)) if t + 1 < n_qt else []
            if t > 0:
                jobs = [lambda st=st: emit_wo(st) for st in range(4 * (t - 1), 4 * t)] + jobs
            if norm1 is not None:
                jobs = [norm1] + jobs
            norm1 = attention_tile(t, jobs)
        norm1()
        for st in range(4 * (n_qt - 1), n_st):
            emit_wo(st)

    _split_excess_waits(nc)
    salt = mybir.InstNoOp(name=f"salt_{_CFG_SALT}", ins=[], outs=[])
    salt.engine = mybir.EngineType.SP
    nc.m.functions[0].blocks[0].instructions.insert(0, salt)
    return nc


_CACHED_NC = None


def _get_nc():
    global _CACHED_NC
    if _CACHED_NC is None:
        _CACHED_NC = build_attention_nc()
    return _CACHED_NC


def _prep_core_inputs(x, mask, w_qkv_w, w_qkv_b, w_o_w, w_o_b, core):
    b = core // 4
    hg = core % 4
    heads = [hg * HPC + h for h in range(HPC)]

    xT = np.ascontiguousarray(x[b].T).astype(NP_BF16)

    def rows(sec, h):  # q=0, k=1, v=2
        base = sec * D + h * DK
        return slice(base, base + DK)

    wqk_rows = np.concatenate(
        [w_qkv_w[rows(1, h)] for h in heads] + [w_qkv_w[rows(0, h)] for h in heads],
        axis=0,
    )  # [512, 1024]
    wqk = np.ascontiguousarray(wqk_rows.T).astype(NP_BF16)

    wv_rows = np.concatenate([w_qkv_w[rows(2, h)] for h in heads], axis=0)
    wv = np.ascontiguousarray(wv_rows.T).astype(NP_BF16)

    wo = np.ascontiguousarray(
        w_o_w[:, hg * HPC * DK:(hg + 1) * HPC * DK].T
    ).astype(NP_BF16)

    bqk = np.concatenate(
        [w_qkv_b[rows(1, h)] for h in heads] + [w_qkv_b[rows(0, h)] for h in heads]
    ).astype(np.float32)[:, None]
    bv = np.concatenate([w_qkv_b[rows(2, h)] for h in heads]).astype(np.float32)[:, None]

    # Causal fringe pattern from the provided mask tensor: for the exact
    # diagonal 128x128 sub-block, [k, q] = mask[q, k] (1 iff q >= k).
    m2d = np.asarray(mask[0, 0])
    fring = np.ascontiguousarray(m2d[0:K_BLK, 0:K_BLK].T.astype(np.float32)).astype(
        NP_BF16
    )

    return {
        "xT": xT, "wqk": wqk, "wv": wv, "wo": wo,
        "bqk": bqk, "bv": bv, "fring": fring,
    }


def kernel(x, mask, w_qkv_w, w_qkv_b, w_o_w, w_o_b, _profile=False):
    x = np.asarray(x, np.float32)
    w_qkv_w = np.asarray(w_qkv_w, np.float32)
    w_qkv_b = np.asarray(w_qkv_b, np.float32)
    w_o_w = np.asarray(w_o_w, np.float32)
    w_o_b = np.asarray(w_o_b, np.float32)

    nc = _get_nc()
    in_maps = [
        _prep_core_inputs(x, mask, w_qkv_w, w_qkv_b, w_o_w, w_o_b, c)
        for c in range(N_CORES)
    ]
    res = run_bass_kernel_spmd(
        nc, in_maps, core_ids=list(range(N_CORES)), trace=_profile
    )
    y = np.zeros((B, S, D), np.float32)
    for c in range(N_CORES):
        y[c // 4] += np.asarray(res.results[c]["y"], np.float32)
    y += w_o_b[None, None, :]
    if _profile:
        return y, res
    return y
